# revision 1
# baseline (speedup 1.0000x reference)
"""BasePNARetriever Trainium2 kernel (8 NeuronCores, SPMD).

Strategy:
  - Vocab-shard the big embedding @ W_down.T matmul: each core streams a
    [4096, 4096] (host-transposed, padded) slice of text_embeddings and
    computes RtabT[64, 4096] on PE, accumulating in PSUM over 32 K-chunks.
  - PE-transpose RtabT back to row-major rloc[4096, 64], AllGather into the
    full rtab[32768, 64] (vocab padded 4000->4096 per shard; token ids are
    host-remapped accordingly -- max id 32671 fits int16).
  - Each core dma_gathers its 6272 (padded) rows x 16 tokens = 100352
    vectors of 256B from rtab, reduces over the 16 tokens (sum/max/min and
    sum-of-squares) with DVE binary trees, finishes mean/std, applies the
    small FC (features[256] x 3 scales -> 64) on PE via PE-transposed
    features, adds bias and L2-normalizes.
  - Host precomputes lengths/log-scales (includes a global mean over all
    50000 rows) and patches the rare rows containing id==0 tokens (the
    device path ignores the valid-token mask; ~25 rows in 50000).

Sync-architecture notes (walrus limits): a Matmult may carry at most ONE
sync wait; HWDGE (nc.sync) DMAs are also tightly limited; DVE/ACT/Pool
queue instructions tolerate several.  Hence: emb chunk loads go through
SWDGE (nc.gpsimd), every PE dummy/absorber reads only wdt_sb (whose DMA
lane PE observes on its first matmul), an extra junk matmul into the psA
pad columns absorbs the PSUM drain wait at the psA->psT transition, and
phase C reuses the still-open psT pool (psF=psT) so PSUM bank history is
already PE-observed when the feature transposes start.
"""

import sys

sys.path.insert(0, "/opt/trn_rl_repo")

import os

import numpy as np

import concourse.bass as bass
import concourse.bacc as bacc
import concourse.mybir as mybir
import concourse.tile as tile
from concourse.bass_utils import run_bass_kernel_spmd

F32 = mybir.dt.float32
I16 = mybir.dt.int16
AF = mybir.ActivationFunctionType
ALU = mybir.AluOpType

NCORES = 8
VOCAB, HID, R, B, S = 32000, 4096, 64, 50000, 16
VSH = VOCAB // NCORES          # 4000 real vocab rows per core
VSHP = 4096                    # padded vocab rows per core (32 x 128)
VOCABP = VSHP * NCORES         # 32768 padded vocab
KC = HID // 128                # 32 contraction chunks
BSH = B // NCORES              # 6250 rows per core
NT = 49                        # row tiles of 128 (6272 padded rows)
BPAD = NT * 128                # 6272
CH_T = 4                       # row-tiles per gather chunk
CHUNKS = [(i, min(CH_T, NT - i)) for i in range(0, NT, CH_T)]  # (tile0, ntiles)


def _phase_c(nc, tc, psT, rtab, idx_sb, aux_sb, wret_sb, biasr_sb, ident2_sb,
             wdt_sb, ostage):
    with (
        tc.tile_pool(name="g", bufs=2) as gpool,
        tc.tile_pool(name="sq", bufs=2) as sqpool,
        tc.tile_pool(name="tr", bufs=2) as tpool,
        tc.tile_pool(name="f", bufs=2) as fpool,
        tc.tile_pool(name="psG", bufs=1, space="PSUM") as psG,
    ):
        psF = psT  # reuse the open pool: bank history already PE-observed

        def tree(src3d, dst, op):
            # src3d: [128, 16, 64] -> dst [128, (1,) 64] reducing dim 1
            a = tpool.tile([128, 8, R], F32, tag="tr8")
            nc.vector.tensor_tensor(a[:], src3d[:, 0:8, :], src3d[:, 8:16, :], op)
            b = tpool.tile([128, 4, R], F32, tag="tr4")
            nc.vector.tensor_tensor(b[:], a[:, 0:4, :], a[:, 4:8, :], op)
            c = tpool.tile([128, 2, R], F32, tag="tr2")
            nc.vector.tensor_tensor(c[:], b[:, 0:2, :], b[:, 2:4, :], op)
            nc.vector.tensor_tensor(dst, c[:, 0:1, :], c[:, 1:2, :], op)

        stage = os.environ.get("KSTAGE", "full")
        for (t0, ntile) in CHUNKS:
            nidx = ntile * 2048
            g = gpool.tile([128, CH_T * 16, R], F32, tag="g")
            if stage in ("gather", "full"):
                nc.gpsimd.dma_gather(
                    g[:, : ntile * 16, :],
                    rtab[:],
                    idx_sb[:, t0 * 128 : t0 * 128 + nidx // 16],
                    nidx,
                    nidx,
                    R,
                    single_packet=False,
                )
            if stage != "full":
                continue
            for tt in range(ntile):
                t = t0 + tt
                gt = g[:, tt * 16 : (tt + 1) * 16, :]
                sq = sqpool.tile([128, 16, R], F32, tag="sq")
                nc.scalar.square(sq[:], gt)

                feat = fpool.tile([128, 256], F32, tag="feat")
                tree(gt, feat[:, 0:64], ALU.add)
                tree(gt, feat[:, 64:128], ALU.max)
                tree(gt, feat[:, 128:192], ALU.min)
                sqs = fpool.tile([128, R], F32, tag="sqs")
                tree(sq, sqs[:], ALU.add)

                invl = aux_sb[:, t : t + 1]
                sc = aux_sb[:, NT + t : NT + t + 1]
                isc = aux_sb[:, 2 * NT + t : 2 * NT + t + 1]

                # mean / std
                nc.vector.tensor_scalar_mul(feat[:, 0:64], feat[:, 0:64], invl)
                sqm = fpool.tile([128, R], F32, tag="sqm")
                nc.vector.tensor_scalar_mul(sqm[:], sqs[:], invl)
                msq = fpool.tile([128, R], F32, tag="msq")
                nc.scalar.square(msq[:], feat[:, 0:64])
                nc.vector.tensor_tensor(sqm[:], sqm[:], msq[:], ALU.subtract)
                nc.vector.tensor_scalar_max(sqm[:], sqm[:], 1e-6)
                # sqrt on ACT, then DVE copy so all feat writers are DVE
                stdt = fpool.tile([128, R], F32, tag="stdt")
                nc.scalar.sqrt(stdt[:], sqm[:])
                nc.vector.tensor_copy(feat[:, 192:256], stdt[:])

                # FC: G_k = features @ W_k.T via PE-transposed features
                fts = []
                for kc in range(2):
                    ftp = psF.tile([128, 128], F32, tag="ftp")
                    nc.tensor.transpose(
                        ftp[:], feat[:, kc * 128 : (kc + 1) * 128], ident2_sb[:]
                    )
                    ft = fpool.tile([128, 128], F32, tag=f"fts{kc}")
                    nc.scalar.activation(ft[:], ftp[:], AF.Copy)
                    fts.append(ft)
                gps = [
                    psG.tile([128, R], F32, tag=f"gp{k}", name=f"gp{k}_{t}")
                    for k in range(3)
                ]
                for kc in range(2):
                    for k in range(3):
                        nc.tensor.matmul(
                            gps[k][:],
                            fts[kc][:],
                            wret_sb[:, kc, k * R : (k + 1) * R],
                            start=(kc == 0),
                            stop=(kc == 1),
                        )
                # res = G0 + G1*scale + G2*iscale + bias   (all PSUM readers DVE)
                r1 = fpool.tile([128, R], F32, tag="r1")
                nc.vector.tensor_scalar_mul(r1[:], gps[1][:], sc)
                r2 = fpool.tile([128, R], F32, tag="r2")
                nc.vector.tensor_scalar_mul(r2[:], gps[2][:], isc)
                res = fpool.tile([128, R], F32, tag="res")
                nc.vector.tensor_add(res[:], gps[0][:], r1[:])
                nc.vector.tensor_add(res[:], res[:], r2[:])
                nc.vector.tensor_add(res[:], res[:], biasr_sb[:])
                # L2 normalize
                sqr = fpool.tile([128, R], F32, tag="sqr")
                nc.scalar.square(sqr[:], res[:])
                ss = fpool.tile([128, 1], F32, tag="ss")
                nc.vector.tensor_reduce(ss[:], sqr[:], mybir.AxisListType.X, ALU.add)
                nrm = fpool.tile([128, 1], F32, tag="nrm")
                nc.scalar.sqrt(nrm[:], ss[:])
                nc.vector.tensor_scalar_max(nrm[:], nrm[:], 1e-12)
                rin = fpool.tile([128, 1], F32, tag="rin")
                nc.vector.reciprocal(rin[:], nrm[:])
                nc.vector.tensor_scalar_mul(ostage[:, t, :], res[:], rin[:])


def build_kernel():
    nc = bacc.Bacc(
        "TRN2",
        target_bir_lowering=False,
        debug=False,
        num_devices=NCORES,
    )
    embt = nc.declare_dram_parameter("embt", [HID, VSHP], F32, isOutput=False)
    wdt = nc.declare_dram_parameter("wdt", [HID, R], F32, isOutput=False)
    idx = nc.declare_dram_parameter("idx", [128, BPAD], I16, isOutput=False)
    aux = nc.declare_dram_parameter("aux", [128, 3 * NT], F32, isOutput=False)
    wret = nc.declare_dram_parameter("wret", [2, 128, 3 * R], F32, isOutput=False)
    biasr = nc.declare_dram_parameter("biasr", [128, R], F32, isOutput=False)
    ident = nc.declare_dram_parameter("ident", [128, 128], F32, isOutput=False)
    out = nc.declare_dram_parameter("out", [BPAD, R], F32, isOutput=True)

    with tile.TileContext(nc) as tc:
        with (
            tc.tile_pool(name="dram", bufs=1, space="DRAM") as dpool,
            tc.tile_pool(name="const", bufs=1) as cpool,
        ):
            rloc = dpool.tile([VSHP, R], F32)
            rtab = dpool.tile([VOCABP, R], F32)

            wdt_sb = cpool.tile([128, KC, R], F32)
            nc.sync.dma_start(wdt_sb[:], wdt.rearrange("(k p) n -> p k n", p=128))
            idx_sb = cpool.tile([128, BPAD], I16)
            nc.sync.dma_start(idx_sb[:], idx[:])
            aux_sb = cpool.tile([128, 3 * NT], F32)
            nc.sync.dma_start(aux_sb[:], aux[:])
            wret_raw = cpool.tile([128, 2, 3 * R], F32)
            nc.sync.dma_start(wret_raw[:], wret.rearrange("c p n -> p c n"))
            wret_sb = cpool.tile([128, 2, 3 * R], F32)
            nc.vector.tensor_copy(wret_sb[:], wret_raw[:])
            biasr_sb = cpool.tile([128, R], F32)
            nc.sync.dma_start(biasr_sb[:], biasr[:])
            ident_sb = cpool.tile([128, 128], F32)
            nc.sync.dma_start(ident_sb[:], ident[:])
            ostage = cpool.tile([128, NT, R], F32)

            # identity staged through DVE so PE transposes dep on DVE sem only
            ident2_sb = cpool.tile([128, 128], F32)
            nc.vector.tensor_copy(ident2_sb[:], ident_sb[:])

            # ---- Phase A: RtabT = W_downT.T @ embT ----
            # KREPS>1 repeats the whole pipeline for launch-overhead-free
            # wall-clock measurement ((T(N)-T(1))/(N-1) = per-rep time).
            for _rep in range(int(os.environ.get("KREPS", "1"))):
              with (
                  tc.tile_pool(name="emb", bufs=2) as epool,
                  tc.tile_pool(name="stageA", bufs=1) as apool,
              ):
                  rtabT_sb = apool.tile([64, VSHP], F32)
                  with tc.tile_pool(name="psA", bufs=1, space="PSUM") as psA:
                      rtabT_ps = psA.tile([64, VSHP], F32)
                      # gate: junk matmul reading only wdt_sb -> absorbs the wdt
                      # DMA-lane wait so real matmuls carry just their ech lane
                      nc.tensor.matmul(
                          rtabT_ps[:, VSHP - 64 : VSHP - 32],
                          wdt_sb[:, 0, :],
                          wdt_sb[:, 0, 0:32],
                          start=True,
                          stop=True,
                          skip_group_check=True,
                      )
                      for k in range(KC):
                          ech = epool.tile([128, VSHP], F32, tag="ech")
                          nc.gpsimd.dma_start(ech[:], embt[k * 128 : (k + 1) * 128, :])
                          for vb in range(VSHP // 512):
                              c0 = vb * 512
                              c1 = min((vb + 1) * 512, VSHP - 64)
                              nc.tensor.matmul(
                                  rtabT_ps[:, c0:c1],
                                  wdt_sb[:, k, :],
                                  ech[:, c0:c1],
                                  start=(k == 0),
                                  stop=(k == KC - 1),
                              )
                      # absorber: junk matmul into the other pad half; its only
                      # wait is the PSUM drain (PE self-sem), freeing later
                      # matmuls from carrying it (Matmult = 1 wait max)
                      nc.tensor.matmul(
                          rtabT_ps[:, VSHP - 32 : VSHP],
                          wdt_sb[:, 0, :],
                          wdt_sb[:, 0, 32:64],
                          start=True,
                          stop=True,
                          skip_group_check=True,
                      )
                      nc.vector.tensor_copy(rtabT_sb[:], rtabT_ps[:])

                  rloc_sb = apool.tile([128, VSHP // 128, R], F32)
                  with tc.tile_pool(name="psT", bufs=2, space="PSUM") as psT:
                      # dummy junk matmul: carries the psA->psT PSUM drain wait
                      dtp = psT.tile([64, 64], F32, tag="tp")
                      nc.tensor.matmul(
                          dtp[:], wdt_sb[:, 0, :], wdt_sb[:, 0, :],
                          start=True, stop=True,
                      )
                      nc.vector.tensor_copy(ostage[0:64, NT - 1, :], dtp[:])
                      for v in range(VSHP // 128):
                          tp = psT.tile([128, 64], F32, tag="tp")
                          nc.tensor.transpose(
                              tp[:],
                              rtabT_sb[:, v * 128 : (v + 1) * 128],
                              ident2_sb[:64, :64],
                          )
                          nc.vector.tensor_copy(rloc_sb[:, v, :], tp[:])
                      nc.sync.dma_start(
                          rloc.rearrange("(v p) n -> p v n", p=128), rloc_sb[:]
                      )

                      # ---- Phase B: AllGather rloc -> rtab ----
                      nc.gpsimd.collective_compute(
                          "AllGather",
                          ALU.bypass,
                          replica_groups=[list(range(NCORES))],
                          ins=[rloc.opt()],
                          outs=[rtab.opt()],
                      )

                      # ---- Phase C: gather + pool + FC ----
                      _phase_c(nc, tc, psT, rtab, idx_sb, aux_sb, wret_sb,
                               biasr_sb, ident2_sb, wdt_sb, ostage)

                      nc.sync.dma_start(
                          out.rearrange("(t p) n -> p t n", p=128), ostage[:]
                      )

    # Bacc's compile pipeline handles wait-limit lowering
    # (move_matmul_waits_to_ldweights, event semaphores, regalloc, ...)
    nc.compile()
    return nc


_NC_CACHE = {}


def _get_nc():
    key = (os.environ.get("KREPS", "1"), os.environ.get("KSTAGE", "full"))
    if key not in _NC_CACHE:
        _NC_CACHE[key] = build_kernel()
    return _NC_CACHE[key]


def _prepare(text_embeddings, kgl2token, W_down, W_re, b_re):
    emb = np.ascontiguousarray(np.asarray(text_embeddings, dtype=np.float32))
    ids = np.asarray(kgl2token)
    wd = np.asarray(W_down, dtype=np.float32)
    wr = np.asarray(W_re, dtype=np.float32)
    br = np.asarray(b_re, dtype=np.float32)

    # host-side scalars: lengths and scale factors (global mean over all rows)
    lengths = (ids > 0).sum(axis=1).astype(np.float32)  # [B]
    scale = np.log(lengths + 0.0)
    scale = scale / (scale.mean() + 1e-10)
    iscale = 1.0 / np.clip(scale, 0.01, None)
    invl = (1.0 / (lengths + 1e-10)).astype(np.float32)

    # remap ids into padded vocab layout
    ids64 = ids.astype(np.int64)
    rid = (ids64 // VSH) * VSHP + (ids64 % VSH)  # [B, S] < 32768

    wdt = np.ascontiguousarray(wd.T)  # [4096, 64]

    # W_re: result index = feat*3 + k  ->  W_k = W_re[:, k::3]  [64, 256]
    wret = np.zeros((2, 128, 3 * R), dtype=np.float32)
    for k in range(3):
        wkT = np.ascontiguousarray(wr[:, k::3].T)  # [256, 64]
        for kc in range(2):
            wret[kc, :, k * R : (k + 1) * R] = wkT[kc * 128 : (kc + 1) * 128, :]
    biasr = np.tile(br[None, :], (128, 1)).astype(np.float32)
    identm = np.eye(128, dtype=np.float32)

    in_maps = []
    for c in range(NCORES):
        embt = np.zeros((HID, VSHP), dtype=np.float32)
        embt[:, :VSH] = emb[c * VSH : (c + 1) * VSH, :].T
        # per-core padded rows
        rid_c = np.zeros((BPAD, S), dtype=np.int64)
        rid_c[:BSH] = rid[c * BSH : (c + 1) * BSH]
        # gather order: j = t*2048 + s*128 + r
        L = rid_c.reshape(NT, 128, S).transpose(0, 2, 1).reshape(-1)  # [BPAD*S]
        idx16 = L.reshape(-1, 16).T.astype(np.int16)  # [16, BPAD]
        idxsb = np.ascontiguousarray(np.tile(idx16, (8, 1)))  # [128, BPAD]

        auxc = np.zeros((128, 3 * NT), dtype=np.float32)
        for name_i, v in enumerate((invl, scale, iscale)):
            vc = np.ones(BPAD, dtype=np.float32)
            vc[:BSH] = v[c * BSH : (c + 1) * BSH]
            auxc[:, name_i * NT : (name_i + 1) * NT] = vc.reshape(NT, 128).T
        in_maps.append(
            dict(embt=embt, wdt=wdt, idx=idxsb, aux=auxc, wret=wret,
                 biasr=biasr, ident=identm)
        )
    return in_maps, lengths, scale, iscale, invl


def _patch_rows(result, text_embeddings, kgl2token, W_down, W_re, b_re,
                scale_all, iscale_all, invl_all):
    """Recompute rows containing any id==0 token exactly (host, numpy)."""
    ids = np.asarray(kgl2token)
    bad = np.nonzero((ids <= 0).any(axis=1))[0]
    if len(bad) == 0:
        return result
    emb = np.asarray(text_embeddings, dtype=np.float32)
    wd = np.asarray(W_down, dtype=np.float32)
    wr = np.asarray(W_re, dtype=np.float32)
    br = np.asarray(b_re, dtype=np.float32)
    for r in bad:
        tok_ids = ids[r].astype(np.int64)
        tok = emb[tok_ids] @ wd.T  # [S, R]
        mask = (tok_ids > 0).astype(np.float32)[:, None]
        length = mask.sum()
        masked = tok * mask
        mean = masked.sum(axis=0) / (length + 1e-10)
        sq_mean = (tok * tok * mask).sum(axis=0) / (length + 1e-10)
        mx = (masked + (1.0 - mask) * (-1e10)).max(axis=0)
        mn = (masked + (1.0 - mask) * (1e10)).min(axis=0)
        std = np.sqrt(np.clip(sq_mean - mean * mean, 1e-6, None))
        features = np.concatenate([mean, mx, mn, std])  # [256]
        scales = np.array([1.0, scale_all[r], iscale_all[r]], dtype=np.float32)
        flat = (features[:, None] * scales[None, :]).reshape(-1)  # [768]
        res = flat @ wr.T + br
        nrm = np.linalg.norm(res)
        result[r] = res / max(nrm, 1e-12)
    return result


def kernel(text_embeddings, kgl2token, W_down, W_re, b_re, _trace=False):
    nc = _get_nc()
    in_maps, lengths, scale, iscale, invl = _prepare(
        text_embeddings, kgl2token, W_down, W_re, b_re
    )
    r = run_bass_kernel_spmd(nc, in_maps, core_ids=list(range(NCORES)), trace=_trace)
    outs = [r.results[c]["out"][:BSH] for c in range(NCORES)]
    result = np.concatenate(outs, axis=0).astype(np.float32)
    result = _patch_rows(
        result, text_embeddings, kgl2token, W_down, W_re, b_re, scale, iscale, invl
    )
    if _trace:
        return result, r
    return result



# revision 5
# speedup vs baseline: 1.1604x; 1.1604x over previous
"""BasePNARetriever Trainium2 kernel (8 NeuronCores, SPMD).

Strategy:
  - Vocab-shard the big embedding @ W_down.T matmul: each core streams a
    [4096, 4096] (host-transposed, padded) slice of text_embeddings and
    computes RtabT[64, 4096] on PE, accumulating in PSUM over 32 K-chunks.
  - PE-transpose RtabT back to row-major rloc[4096, 64], AllGather into the
    full rtab[32768, 64] (vocab padded 4000->4096 per shard; token ids are
    host-remapped accordingly -- max id 32671 fits int16).
  - Each core dma_gathers its 6272 (padded) rows x 16 tokens = 100352
    vectors of 256B from rtab, reduces over the 16 tokens (sum/max/min and
    sum-of-squares) with DVE binary trees, finishes mean/std, applies the
    small FC (features[256] x 3 scales -> 64) on PE via PE-transposed
    features, adds bias and L2-normalizes.
  - Host precomputes lengths/log-scales (includes a global mean over all
    50000 rows) and patches the rare rows containing id==0 tokens (the
    device path ignores the valid-token mask; ~25 rows in 50000).

Sync-architecture notes (walrus limits): a Matmult may carry at most ONE
sync wait; HWDGE (nc.sync) DMAs are also tightly limited; DVE/ACT/Pool
queue instructions tolerate several.  Hence: emb chunk loads go through
SWDGE (nc.gpsimd), every PE dummy/absorber reads only wdt_sb (whose DMA
lane PE observes on its first matmul), an extra junk matmul into the psA
pad columns absorbs the PSUM drain wait at the psA->psT transition, and
phase C reuses the still-open psT pool (psF=psT) so PSUM bank history is
already PE-observed when the feature transposes start.
"""

import sys

sys.path.insert(0, "/opt/trn_rl_repo")

import os

import numpy as np

import concourse.bass as bass
import concourse.bacc as bacc
import concourse.mybir as mybir
import concourse.tile as tile
from concourse.bass_utils import run_bass_kernel_spmd

F32 = mybir.dt.float32
I16 = mybir.dt.int16
AF = mybir.ActivationFunctionType
ALU = mybir.AluOpType

NCORES = 8
VOCAB, HID, R, B, S = 32000, 4096, 64, 50000, 16
VSH = VOCAB // NCORES          # 4000 real vocab rows per core
VSHP = 4096                    # padded vocab rows per core (32 x 128)
VOCABP = VSHP * NCORES         # 32768 padded vocab
KC = HID // 128                # 32 contraction chunks
BSH = B // NCORES              # 6250 rows per core
NT = 49                        # row tiles of 128 (6272 padded rows)
BPAD = NT * 128                # 6272
CH_T = 4                       # row-tiles per gather chunk
CHUNKS = [(i, min(CH_T, NT - i)) for i in range(0, NT, CH_T)]  # (tile0, ntiles)


def _phase_c(nc, tc, psT, rtab, idx_sb, aux_sb, wret_sb, biasr_sb, ident2_sb,
             wdt_sb, ostage):
    with (
        tc.tile_pool(name="g", bufs=4) as gpool,
        tc.tile_pool(name="sq", bufs=2) as sqpool,
        tc.tile_pool(name="tr", bufs=2) as tpool,
        tc.tile_pool(name="f", bufs=2) as fpool,
        tc.tile_pool(name="psG", bufs=1, space="PSUM") as psG,
    ):
        psF = psT  # reuse the open pool: bank history already PE-observed

        def tree(src3d, dst, op):
            # src3d: [128, 16, 64] -> dst [128, (1,) 64] reducing dim 1
            a = tpool.tile([128, 8, R], F32, tag="tr8")
            nc.vector.tensor_tensor(a[:], src3d[:, 0:8, :], src3d[:, 8:16, :], op)
            b = tpool.tile([128, 4, R], F32, tag="tr4")
            nc.vector.tensor_tensor(b[:], a[:, 0:4, :], a[:, 4:8, :], op)
            c = tpool.tile([128, 2, R], F32, tag="tr2")
            nc.vector.tensor_tensor(c[:], b[:, 0:2, :], b[:, 2:4, :], op)
            nc.vector.tensor_tensor(dst, c[:, 0:1, :], c[:, 1:2, :], op)

        stage = os.environ.get("KSTAGE", "full")
        for ci, (t0, ntile) in enumerate(CHUNKS):
            nidx = ntile * 2048
            g = gpool.tile([128, CH_T * 16, R], F32, tag="g")
            if stage in ("gather", "full"):
                nc.gpsimd.dma_gather(
                    g[:, : ntile * 16, :],
                    rtab[:],
                    idx_sb[:, t0 * 128 : t0 * 128 + nidx // 16],
                    nidx,
                    nidx,
                    R,
                    single_packet=False,
                    queue_num=ci % 4,
                )
            if stage != "full":
                continue
            for tt in range(ntile):
                t = t0 + tt
                gt = g[:, tt * 16 : (tt + 1) * 16, :]
                sq = sqpool.tile([128, 16, R], F32, tag="sq")
                nc.scalar.square(sq[:], gt)

                feat = fpool.tile([128, 256], F32, tag="feat")
                tree(gt, feat[:, 0:64], ALU.add)
                tree(gt, feat[:, 64:128], ALU.max)
                tree(gt, feat[:, 128:192], ALU.min)
                sqs = fpool.tile([128, R], F32, tag="sqs")
                tree(sq, sqs[:], ALU.add)

                invl = aux_sb[:, t : t + 1]
                sc = aux_sb[:, NT + t : NT + t + 1]
                isc = aux_sb[:, 2 * NT + t : 2 * NT + t + 1]

                # mean / std
                nc.vector.tensor_scalar_mul(feat[:, 0:64], feat[:, 0:64], invl)
                sqm = fpool.tile([128, R], F32, tag="sqm")
                nc.vector.tensor_scalar_mul(sqm[:], sqs[:], invl)
                msq = fpool.tile([128, R], F32, tag="msq")
                nc.scalar.square(msq[:], feat[:, 0:64])
                nc.vector.tensor_tensor(sqm[:], sqm[:], msq[:], ALU.subtract)
                nc.vector.tensor_scalar_max(sqm[:], sqm[:], 1e-6)
                # sqrt on ACT, then DVE copy so all feat writers are DVE
                stdt = fpool.tile([128, R], F32, tag="stdt")
                nc.scalar.sqrt(stdt[:], sqm[:])
                nc.vector.tensor_copy(feat[:, 192:256], stdt[:])

                # FC: G_k = features @ W_k.T via PE-transposed features
                fts = []
                for kc in range(2):
                    ftp = psF.tile([128, 128], F32, tag="ftp")
                    nc.tensor.transpose(
                        ftp[:], feat[:, kc * 128 : (kc + 1) * 128], ident2_sb[:]
                    )
                    ft = fpool.tile([128, 128], F32, tag=f"fts{kc}")
                    nc.scalar.activation(ft[:], ftp[:], AF.Copy)
                    fts.append(ft)
                gps = [
                    psG.tile([128, R], F32, tag=f"gp{k}", name=f"gp{k}_{t}")
                    for k in range(3)
                ]
                for kc in range(2):
                    for k in range(3):
                        nc.tensor.matmul(
                            gps[k][:],
                            fts[kc][:],
                            wret_sb[:, kc, k * R : (k + 1) * R],
                            start=(kc == 0),
                            stop=(kc == 1),
                        )
                # res = G0 + G1*scale + G2*iscale + bias   (all PSUM readers DVE)
                r1 = fpool.tile([128, R], F32, tag="r1")
                nc.vector.tensor_scalar_mul(r1[:], gps[1][:], sc)
                r2 = fpool.tile([128, R], F32, tag="r2")
                nc.vector.tensor_scalar_mul(r2[:], gps[2][:], isc)
                res = fpool.tile([128, R], F32, tag="res")
                nc.vector.tensor_add(res[:], gps[0][:], r1[:])
                nc.vector.tensor_add(res[:], res[:], r2[:])
                nc.vector.tensor_add(res[:], res[:], biasr_sb[:])
                # L2 normalize
                sqr = fpool.tile([128, R], F32, tag="sqr")
                nc.scalar.square(sqr[:], res[:])
                ss = fpool.tile([128, 1], F32, tag="ss")
                nc.vector.tensor_reduce(ss[:], sqr[:], mybir.AxisListType.X, ALU.add)
                nrm = fpool.tile([128, 1], F32, tag="nrm")
                nc.scalar.sqrt(nrm[:], ss[:])
                nc.vector.tensor_scalar_max(nrm[:], nrm[:], 1e-12)
                rin = fpool.tile([128, 1], F32, tag="rin")
                nc.vector.reciprocal(rin[:], nrm[:])
                nc.vector.tensor_scalar_mul(ostage[:, t, :], res[:], rin[:])


def build_kernel():
    nc = bacc.Bacc(
        "TRN2",
        target_bir_lowering=False,
        debug=False,
        num_devices=NCORES,
        num_swdge_queues=4,
    )
    embt = nc.declare_dram_parameter("embt", [HID, VSHP], F32, isOutput=False)
    wdt = nc.declare_dram_parameter("wdt", [HID, R], F32, isOutput=False)
    idx = nc.declare_dram_parameter("idx", [128, BPAD], I16, isOutput=False)
    aux = nc.declare_dram_parameter("aux", [128, 3 * NT], F32, isOutput=False)
    wret = nc.declare_dram_parameter("wret", [2, 128, 3 * R], F32, isOutput=False)
    biasr = nc.declare_dram_parameter("biasr", [128, R], F32, isOutput=False)
    ident = nc.declare_dram_parameter("ident", [128, 128], F32, isOutput=False)
    out = nc.declare_dram_parameter("out", [BPAD, R], F32, isOutput=True)

    with tile.TileContext(nc) as tc:
        with (
            tc.tile_pool(name="dram", bufs=1, space="DRAM") as dpool,
            tc.tile_pool(name="const", bufs=1) as cpool,
        ):
            rloc = dpool.tile([VSHP, R], F32)
            rtab = dpool.tile([VOCABP, R], F32)

            wdt_sb = cpool.tile([128, KC, R], F32)
            nc.sync.dma_start(wdt_sb[:], wdt.rearrange("(k p) n -> p k n", p=128))
            idx_sb = cpool.tile([128, BPAD], I16)
            nc.sync.dma_start(idx_sb[:], idx[:])
            aux_sb = cpool.tile([128, 3 * NT], F32)
            nc.sync.dma_start(aux_sb[:], aux[:])
            wret_raw = cpool.tile([128, 2, 3 * R], F32)
            nc.sync.dma_start(wret_raw[:], wret.rearrange("c p n -> p c n"))
            wret_sb = cpool.tile([128, 2, 3 * R], F32)
            nc.vector.tensor_copy(wret_sb[:], wret_raw[:])
            biasr_sb = cpool.tile([128, R], F32)
            nc.sync.dma_start(biasr_sb[:], biasr[:])
            ident_sb = cpool.tile([128, 128], F32)
            nc.sync.dma_start(ident_sb[:], ident[:])
            ostage = cpool.tile([128, NT, R], F32)

            # identity staged through DVE so PE transposes dep on DVE sem only
            ident2_sb = cpool.tile([128, 128], F32)
            nc.vector.tensor_copy(ident2_sb[:], ident_sb[:])

            # ---- Phase A: RtabT = W_downT.T @ embT ----
            # KREPS>1 repeats the whole pipeline for launch-overhead-free
            # wall-clock measurement ((T(N)-T(1))/(N-1) = per-rep time).
            for _rep in range(int(os.environ.get("KREPS", "1"))):
              with (
                  tc.tile_pool(name="emb", bufs=3) as epool,
                  tc.tile_pool(name="stageA", bufs=1) as apool,
              ):
                  rtabT_sb = apool.tile([64, VSHP], F32)
                  with tc.tile_pool(name="psA", bufs=1, space="PSUM") as psA:
                      rtabT_ps = psA.tile([64, VSHP], F32)
                      # gate: junk matmul reading only wdt_sb -> absorbs the wdt
                      # DMA-lane wait so real matmuls carry just their ech lane
                      nc.tensor.matmul(
                          rtabT_ps[:, VSHP - 64 : VSHP - 32],
                          wdt_sb[:, 0, :],
                          wdt_sb[:, 0, 0:32],
                          start=True,
                          stop=True,
                          skip_group_check=True,
                      )
                      for k in range(KC):
                          ech = epool.tile([128, VSHP], F32, tag="ech")
                          nc.gpsimd.dma_start(ech[:], embt[k * 128 : (k + 1) * 128, :])
                          for vb in range(VSHP // 512):
                              c0 = vb * 512
                              c1 = min((vb + 1) * 512, VSHP - 64)
                              nc.tensor.matmul(
                                  rtabT_ps[:, c0:c1],
                                  wdt_sb[:, k, :],
                                  ech[:, c0:c1],
                                  start=(k == 0),
                                  stop=(k == KC - 1),
                              )
                      # absorber: junk matmul into the other pad half; its only
                      # wait is the PSUM drain (PE self-sem), freeing later
                      # matmuls from carrying it (Matmult = 1 wait max)
                      nc.tensor.matmul(
                          rtabT_ps[:, VSHP - 32 : VSHP],
                          wdt_sb[:, 0, :],
                          wdt_sb[:, 0, 32:64],
                          start=True,
                          stop=True,
                          skip_group_check=True,
                      )
                      nc.vector.tensor_copy(rtabT_sb[:], rtabT_ps[:])

                  rloc_sb = apool.tile([128, VSHP // 128, R], F32)
                  with tc.tile_pool(name="psT", bufs=2, space="PSUM") as psT:
                      # dummy junk matmul: carries the psA->psT PSUM drain wait
                      dtp = psT.tile([64, 64], F32, tag="tp")
                      nc.tensor.matmul(
                          dtp[:], wdt_sb[:, 0, :], wdt_sb[:, 0, :],
                          start=True, stop=True,
                      )
                      nc.vector.tensor_copy(ostage[0:64, NT - 1, :], dtp[:])
                      for v in range(VSHP // 128):
                          tp = psT.tile([128, 64], F32, tag="tp")
                          nc.tensor.transpose(
                              tp[:],
                              rtabT_sb[:, v * 128 : (v + 1) * 128],
                              ident2_sb[:64, :64],
                          )
                          nc.vector.tensor_copy(rloc_sb[:, v, :], tp[:])
                      nc.sync.dma_start(
                          rloc.rearrange("(v p) n -> p v n", p=128), rloc_sb[:]
                      )

                      # ---- Phase B: AllGather rloc -> rtab ----
                      nc.gpsimd.collective_compute(
                          "AllGather",
                          ALU.bypass,
                          replica_groups=[list(range(NCORES))],
                          ins=[rloc.opt()],
                          outs=[rtab.opt()],
                      )

                      # ---- Phase C: gather + pool + FC ----
                      _phase_c(nc, tc, psT, rtab, idx_sb, aux_sb, wret_sb,
                               biasr_sb, ident2_sb, wdt_sb, ostage)

                      nc.sync.dma_start(
                          out.rearrange("(t p) n -> p t n", p=128), ostage[:]
                      )

    # Bacc's compile pipeline handles wait-limit lowering
    # (move_matmul_waits_to_ldweights, event semaphores, regalloc, ...)
    nc.compile()
    return nc


_NC_CACHE = {}


def _get_nc():
    key = (os.environ.get("KREPS", "1"), os.environ.get("KSTAGE", "full"))
    if key not in _NC_CACHE:
        _NC_CACHE[key] = build_kernel()
    return _NC_CACHE[key]


def _prepare(text_embeddings, kgl2token, W_down, W_re, b_re):
    emb = np.ascontiguousarray(np.asarray(text_embeddings, dtype=np.float32))
    ids = np.asarray(kgl2token)
    wd = np.asarray(W_down, dtype=np.float32)
    wr = np.asarray(W_re, dtype=np.float32)
    br = np.asarray(b_re, dtype=np.float32)

    # host-side scalars: lengths and scale factors (global mean over all rows)
    lengths = (ids > 0).sum(axis=1).astype(np.float32)  # [B]
    scale = np.log(lengths + 0.0)
    scale = scale / (scale.mean() + 1e-10)
    iscale = 1.0 / np.clip(scale, 0.01, None)
    invl = (1.0 / (lengths + 1e-10)).astype(np.float32)

    # remap ids into padded vocab layout
    ids64 = ids.astype(np.int64)
    rid = (ids64 // VSH) * VSHP + (ids64 % VSH)  # [B, S] < 32768

    wdt = np.ascontiguousarray(wd.T)  # [4096, 64]

    # W_re: result index = feat*3 + k  ->  W_k = W_re[:, k::3]  [64, 256]
    wret = np.zeros((2, 128, 3 * R), dtype=np.float32)
    for k in range(3):
        wkT = np.ascontiguousarray(wr[:, k::3].T)  # [256, 64]
        for kc in range(2):
            wret[kc, :, k * R : (k + 1) * R] = wkT[kc * 128 : (kc + 1) * 128, :]
    biasr = np.tile(br[None, :], (128, 1)).astype(np.float32)
    identm = np.eye(128, dtype=np.float32)

    in_maps = []
    for c in range(NCORES):
        embt = np.zeros((HID, VSHP), dtype=np.float32)
        embt[:, :VSH] = emb[c * VSH : (c + 1) * VSH, :].T
        # per-core padded rows
        rid_c = np.zeros((BPAD, S), dtype=np.int64)
        rid_c[:BSH] = rid[c * BSH : (c + 1) * BSH]
        # gather order: j = t*2048 + s*128 + r
        L = rid_c.reshape(NT, 128, S).transpose(0, 2, 1).reshape(-1)  # [BPAD*S]
        idx16 = L.reshape(-1, 16).T.astype(np.int16)  # [16, BPAD]
        idxsb = np.ascontiguousarray(np.tile(idx16, (8, 1)))  # [128, BPAD]

        auxc = np.zeros((128, 3 * NT), dtype=np.float32)
        for name_i, v in enumerate((invl, scale, iscale)):
            vc = np.ones(BPAD, dtype=np.float32)
            vc[:BSH] = v[c * BSH : (c + 1) * BSH]
            auxc[:, name_i * NT : (name_i + 1) * NT] = vc.reshape(NT, 128).T
        in_maps.append(
            dict(embt=embt, wdt=wdt, idx=idxsb, aux=auxc, wret=wret,
                 biasr=biasr, ident=identm)
        )
    return in_maps, lengths, scale, iscale, invl


def _patch_rows(result, text_embeddings, kgl2token, W_down, W_re, b_re,
                scale_all, iscale_all, invl_all):
    """Recompute rows containing any id==0 token exactly (host, numpy)."""
    ids = np.asarray(kgl2token)
    bad = np.nonzero((ids <= 0).any(axis=1))[0]
    if len(bad) == 0:
        return result
    emb = np.asarray(text_embeddings, dtype=np.float32)
    wd = np.asarray(W_down, dtype=np.float32)
    wr = np.asarray(W_re, dtype=np.float32)
    br = np.asarray(b_re, dtype=np.float32)
    for r in bad:
        tok_ids = ids[r].astype(np.int64)
        tok = emb[tok_ids] @ wd.T  # [S, R]
        mask = (tok_ids > 0).astype(np.float32)[:, None]
        length = mask.sum()
        masked = tok * mask
        mean = masked.sum(axis=0) / (length + 1e-10)
        sq_mean = (tok * tok * mask).sum(axis=0) / (length + 1e-10)
        mx = (masked + (1.0 - mask) * (-1e10)).max(axis=0)
        mn = (masked + (1.0 - mask) * (1e10)).min(axis=0)
        std = np.sqrt(np.clip(sq_mean - mean * mean, 1e-6, None))
        features = np.concatenate([mean, mx, mn, std])  # [256]
        scales = np.array([1.0, scale_all[r], iscale_all[r]], dtype=np.float32)
        flat = (features[:, None] * scales[None, :]).reshape(-1)  # [768]
        res = flat @ wr.T + br
        nrm = np.linalg.norm(res)
        result[r] = res / max(nrm, 1e-12)
    return result


def kernel(text_embeddings, kgl2token, W_down, W_re, b_re, _trace=False):
    nc = _get_nc()
    in_maps, lengths, scale, iscale, invl = _prepare(
        text_embeddings, kgl2token, W_down, W_re, b_re
    )
    r = run_bass_kernel_spmd(nc, in_maps, core_ids=list(range(NCORES)), trace=_trace)
    outs = [r.results[c]["out"][:BSH] for c in range(NCORES)]
    result = np.concatenate(outs, axis=0).astype(np.float32)
    result = _patch_rows(
        result, text_embeddings, kgl2token, W_down, W_re, b_re, scale, iscale, invl
    )
    if _trace:
        return result, r
    return result



# revision 9
# speedup vs baseline: 1.3527x; 1.1657x over previous
"""BasePNARetriever Trainium2 kernel (8 NeuronCores, SPMD).

Strategy (v2):
  - Vocab-shard the big embedding @ W_down.T matmul: each core streams a
    [4096, 4096] (host-transposed, padded) slice of text_embeddings and
    computes RtabT[64, 4096] on PE, accumulating in PSUM over 32 K-chunks.
  - PE-transpose RtabT back to row-major and emit a bf16 table with each
    row packed as [val(64) | val^2(64)] (256B rows): the squares ride along
    for free in the gather, eliminating the per-chunk ACT square pass.
    AllGather the 1MB local slice into the full rtab2[32768, 128] bf16.
  - Each core dma_gathers its (padded) rows x 16 tokens = 100352 vectors of
    256B from rtab2, round-robining the 4 SWDGE queues so descriptor
    generation runs on all four Q7 core-pairs concurrently (the Q7 desc-gen
    at ~8ns/desc is otherwise the phase C wall).  Pooling runs as chunk-wide
    bf16 binary trees on DVE (sum+sumsq fused over the packed 128-elem
    rows), per-row scalar finals split between ACT (scale-activations) and
    DVE, small FC on PE via PE-transposed bf16 features, L2-normalize.
  - Host precomputes lengths/log-scales (includes a global mean over all
    50000 rows) and patches the rare rows containing id==0 tokens (the
    device path ignores the valid-token mask; ~25 rows in 50000).

Sync-architecture notes (walrus limits): a Matmult may carry at most ONE
sync wait; HWDGE (nc.sync) DMAs are also tightly limited; DVE/ACT/Pool
queue instructions tolerate several.  Hence: emb chunk loads go through
SWDGE (nc.gpsimd), every PE dummy/absorber reads only wdt_sb (whose DMA
lane PE observes on its first matmul), an extra junk matmul into the psA
pad columns absorbs the PSUM drain wait at the psA->psT transition, and
phase C reuses the still-open psT pool (psF=psT) so PSUM bank history is
already PE-observed when the feature transposes start.
"""

import sys

sys.path.insert(0, "/opt/trn_rl_repo")

import os

import numpy as np

import concourse.bass as bass
import concourse.bacc as bacc
import concourse.mybir as mybir
import concourse.tile as tile
from concourse.bass_utils import run_bass_kernel_spmd

F32 = mybir.dt.float32
BF = mybir.dt.bfloat16
I16 = mybir.dt.int16
AF = mybir.ActivationFunctionType
ALU = mybir.AluOpType

NCORES = 8
VOCAB, HID, R, B, S = 32000, 4096, 64, 50000, 16
VSH = VOCAB // NCORES          # 4000 real vocab rows per core
VSHP = 4096                    # padded vocab rows per core (32 x 128)
VOCABP = VSHP * NCORES         # 32768 padded vocab
KC = HID // 128                # 32 contraction chunks
BSH = B // NCORES              # 6250 rows per core
NT = 49                        # row tiles of 128 (6272 padded rows)
BPAD = NT * 128                # 6272
E2 = 2 * R                     # 128: packed table row [val(64)|sq(64)]
CH_T = 4                       # row-tiles per gather chunk
CHUNKS = [(i, min(CH_T, NT - i)) for i in range(0, NT, CH_T)]  # (tile0, ntiles)


def _phase_c(nc, tc, psT, rtab2, idx_sb, aux_sb, wret_sb, biasr_sb, identb_sb,
             wdt_sb, ostage):
    with (
        tc.tile_pool(name="g", bufs=3) as gpool,
        tc.tile_pool(name="tr", bufs=1) as tpool,
        tc.tile_pool(name="f", bufs=2) as fpool,
        tc.tile_pool(name="psG", bufs=1, space="PSUM") as psG,
    ):
        psF = psT  # reuse the open pool: bank history already PE-observed

        stage = os.environ.get("KSTAGE", "full")
        for ci, (t0, ntile) in enumerate(CHUNKS):
            nidx = ntile * 2048
            nslot = ntile * 16
            g = gpool.tile([128, CH_T * 16, E2], BF, tag="g")
            if stage in ("gather", "full"):
                nc.gpsimd.dma_gather(
                    g[:, :nslot, :],
                    rtab2[:],
                    idx_sb[:, t0 * 128 : t0 * 128 + nidx // 16],
                    nidx,
                    nidx,
                    E2,
                    single_packet=False,
                    queue_num=ci % 4,
                )
            if stage != "full":
                continue

            g4 = g[:, :nslot, :].rearrange("p (t s) e -> p t s e", s=16)

            # -- chunk-wide bf16 trees over the 16 tokens --
            # add-tree runs on the full packed rows: sums values AND squares
            a1 = tpool.tile([128, CH_T, 8, E2], BF, tag="a1")
            nc.vector.tensor_tensor(
                a1[:, :ntile], g4[:, :, 0:8, :], g4[:, :, 8:16, :], ALU.add)
            a2 = tpool.tile([128, CH_T, 4, E2], BF, tag="a2")
            nc.vector.tensor_tensor(
                a2[:, :ntile], a1[:, :ntile, 0:4], a1[:, :ntile, 4:8], ALU.add)
            a3 = tpool.tile([128, CH_T, 2, E2], BF, tag="a3")
            nc.vector.tensor_tensor(
                a3[:, :ntile], a2[:, :ntile, 0:2], a2[:, :ntile, 2:4], ALU.add)
            addf = tpool.tile([128, CH_T, E2], F32, tag="addf")
            nc.vector.tensor_tensor(
                addf[:, :ntile], a3[:, :ntile, 0], a3[:, :ntile, 1], ALU.add)

            featc = fpool.tile([128, CH_T, 4 * R], BF, tag="featc")

            def mmtree(op, dst_lo):
                b1 = tpool.tile([128, CH_T, 8, R], BF, tag=f"b1{dst_lo}")
                nc.vector.tensor_tensor(
                    b1[:, :ntile], g4[:, :, 0:8, 0:R], g4[:, :, 8:16, 0:R], op)
                b2 = tpool.tile([128, CH_T, 4, R], BF, tag=f"b2{dst_lo}")
                nc.vector.tensor_tensor(
                    b2[:, :ntile], b1[:, :ntile, 0:4], b1[:, :ntile, 4:8], op)
                b3 = tpool.tile([128, CH_T, 2, R], BF, tag=f"b3{dst_lo}")
                nc.vector.tensor_tensor(
                    b3[:, :ntile], b2[:, :ntile, 0:2], b2[:, :ntile, 2:4], op)
                nc.vector.tensor_tensor(
                    featc[:, :ntile, dst_lo : dst_lo + R],
                    b3[:, :ntile, 0], b3[:, :ntile, 1], op)

            mmtree(ALU.max, R)       # max -> featc[:, :, 64:128]
            mmtree(ALU.min, 2 * R)   # min -> featc[:, :, 128:192]

            ss = fpool.tile([128, CH_T], F32, tag="ss")

            for tt in range(ntile):
                t = t0 + tt
                invl = aux_sb[:, t : t + 1]
                sc = aux_sb[:, NT + t : NT + t + 1]
                isc = aux_sb[:, 2 * NT + t : 2 * NT + t + 1]

                # mean / std (fp32 chain on ACT + DVE)
                m = fpool.tile([128, R], F32, tag="m")
                nc.scalar.activation(m[:], addf[:, tt, 0:R], AF.Copy, scale=invl)
                nc.scalar.copy(featc[:, tt, 0:R], m[:])
                sqm = fpool.tile([128, R], F32, tag="sqm")
                nc.scalar.activation(
                    sqm[:], addf[:, tt, R:E2], AF.Copy, scale=invl)
                msq = fpool.tile([128, R], F32, tag="msq")
                nc.scalar.square(msq[:], m[:])
                nc.vector.tensor_tensor(sqm[:], sqm[:], msq[:], ALU.subtract)
                nc.vector.tensor_scalar_max(sqm[:], sqm[:], 1e-6)
                nc.scalar.sqrt(featc[:, tt, 3 * R : 4 * R], sqm[:])

                # FC: G_k = features @ W_k.T via PE-transposed features
                fts = []
                for kc in range(2):
                    ftp = psF.tile([128, 128], BF, tag="ftp")
                    nc.tensor.transpose(
                        ftp[:], featc[:, tt, kc * 128 : (kc + 1) * 128],
                        identb_sb[:],
                    )
                    ft = fpool.tile([128, 128], BF, tag=f"fts{kc}")
                    nc.scalar.activation(ft[:], ftp[:], AF.Copy)
                    fts.append(ft)
                gps = [
                    psG.tile([128, R], F32, tag=f"gp{k}", name=f"gp{k}_{t}")
                    for k in range(3)
                ]
                for kc in range(2):
                    for k in range(3):
                        nc.tensor.matmul(
                            gps[k][:],
                            fts[kc][:],
                            wret_sb[:, kc, k * R : (k + 1) * R],
                            start=(kc == 0),
                            stop=(kc == 1),
                        )
                # res = G0 + G1*scale + G2*iscale + bias
                r1 = fpool.tile([128, R], F32, tag="r1")
                nc.scalar.activation(r1[:], gps[1][:], AF.Copy, scale=sc)
                r2 = fpool.tile([128, R], F32, tag="r2")
                nc.scalar.activation(r2[:], gps[2][:], AF.Copy, scale=isc)
                res = fpool.tile([128, R], F32, tag="res")
                nc.vector.tensor_add(res[:], gps[0][:], r1[:])
                nc.vector.tensor_add(res[:], res[:], r2[:])
                nc.vector.tensor_add(ostage[:, t, :], res[:], biasr_sb[:])
                # L2 norm: squares + row-sum in one ACT op
                sqr = fpool.tile([128, R], F32, tag="sqr")
                nc.scalar.activation(
                    sqr[:], ostage[:, t, :], AF.Square,
                    accum_out=ss[:, tt : tt + 1])

            # normalize per chunk: ostage[:, t, :] *= 1/max(sqrt(ss), eps)
            nrm = fpool.tile([128, CH_T], F32, tag="nrm")
            nc.scalar.sqrt(nrm[:, :ntile], ss[:, :ntile])
            nc.vector.tensor_scalar_max(nrm[:, :ntile], nrm[:, :ntile], 1e-12)
            rin = fpool.tile([128, CH_T], F32, tag="rin")
            nc.vector.reciprocal(rin[:, :ntile], nrm[:, :ntile])
            for tt in range(ntile):
                t = t0 + tt
                nc.vector.tensor_scalar_mul(
                    ostage[:, t, :], ostage[:, t, :], rin[:, tt : tt + 1])


def build_kernel():
    nc = bacc.Bacc(
        "TRN2",
        target_bir_lowering=False,
        debug=False,
        num_devices=NCORES,
        num_swdge_queues=4,
    )
    embt = nc.declare_dram_parameter("embt", [HID, VSHP], F32, isOutput=False)
    wdt = nc.declare_dram_parameter("wdt", [HID, R], F32, isOutput=False)
    idx = nc.declare_dram_parameter("idx", [128, BPAD], I16, isOutput=False)
    aux = nc.declare_dram_parameter("aux", [128, 3 * NT], F32, isOutput=False)
    wret = nc.declare_dram_parameter("wret", [2, 128, 3 * R], BF, isOutput=False)
    biasr = nc.declare_dram_parameter("biasr", [128, R], F32, isOutput=False)
    ident = nc.declare_dram_parameter("ident", [128, 128], F32, isOutput=False)
    out = nc.declare_dram_parameter("out", [BPAD, R], F32, isOutput=True)

    with tile.TileContext(nc) as tc:
        with (
            tc.tile_pool(name="dram", bufs=1, space="DRAM") as dpool,
            tc.tile_pool(name="const", bufs=1) as cpool,
        ):
            rloc2 = dpool.tile([VSHP, E2], BF)
            rtab2 = dpool.tile([VOCABP, E2], BF)

            wdt_sb = cpool.tile([128, KC, R], F32)
            nc.sync.dma_start(wdt_sb[:], wdt.rearrange("(k p) n -> p k n", p=128))
            idx_sb = cpool.tile([128, BPAD], I16)
            nc.sync.dma_start(idx_sb[:], idx[:])
            aux_sb = cpool.tile([128, 3 * NT], F32)
            nc.sync.dma_start(aux_sb[:], aux[:])
            wret_raw = cpool.tile([128, 2, 3 * R], BF)
            nc.sync.dma_start(wret_raw[:], wret.rearrange("c p n -> p c n"))
            wret_sb = cpool.tile([128, 2, 3 * R], BF)
            nc.vector.tensor_copy(wret_sb[:], wret_raw[:])
            biasr_sb = cpool.tile([128, R], F32)
            nc.sync.dma_start(biasr_sb[:], biasr[:])
            ident_sb = cpool.tile([128, 128], F32)
            nc.sync.dma_start(ident_sb[:], ident[:])
            ostage = cpool.tile([128, NT, R], F32)

            # identity staged through DVE so PE transposes dep on DVE sem only
            ident2_sb = cpool.tile([128, 128], F32)
            nc.vector.tensor_copy(ident2_sb[:], ident_sb[:])
            identb_sb = cpool.tile([128, 128], BF)
            nc.vector.tensor_copy(identb_sb[:], ident_sb[:])

            # ---- Phase A: RtabT = W_downT.T @ embT ----
            # KREPS>1 repeats the whole pipeline for launch-overhead-free
            # wall-clock measurement ((T(N)-T(1))/(N-1) = per-rep time).
            for _rep in range(int(os.environ.get("KREPS", "1"))):
              with (
                  tc.tile_pool(name="emb", bufs=3) as epool,
                  tc.tile_pool(name="stageA", bufs=1) as apool,
              ):
                  rtabT_sb = apool.tile([64, VSHP], F32)
                  with tc.tile_pool(name="psA", bufs=1, space="PSUM") as psA:
                      rtabT_ps = psA.tile([64, VSHP], F32)
                      # gate: junk matmul reading only wdt_sb -> absorbs the wdt
                      # DMA-lane wait so real matmuls carry just their ech lane
                      nc.tensor.matmul(
                          rtabT_ps[:, VSHP - 64 : VSHP - 32],
                          wdt_sb[:, 0, :],
                          wdt_sb[:, 0, 0:32],
                          start=True,
                          stop=True,
                          skip_group_check=True,
                      )
                      for k in range(KC):
                          ech = epool.tile([128, VSHP], F32, tag="ech")
                          nc.gpsimd.dma_start(ech[:], embt[k * 128 : (k + 1) * 128, :])
                          for vb in range(VSHP // 512):
                              c0 = vb * 512
                              c1 = min((vb + 1) * 512, VSHP - 64)
                              nc.tensor.matmul(
                                  rtabT_ps[:, c0:c1],
                                  wdt_sb[:, k, :],
                                  ech[:, c0:c1],
                                  start=(k == 0),
                                  stop=(k == KC - 1),
                              )
                      # absorber: junk matmul into the other pad half; its only
                      # wait is the PSUM drain (PE self-sem), freeing later
                      # matmuls from carrying it (Matmult = 1 wait max)
                      nc.tensor.matmul(
                          rtabT_ps[:, VSHP - 32 : VSHP],
                          wdt_sb[:, 0, :],
                          wdt_sb[:, 0, 32:64],
                          start=True,
                          stop=True,
                          skip_group_check=True,
                      )
                      nc.vector.tensor_copy(rtabT_sb[:], rtabT_ps[:])

                  # bf16 table slice, rows packed [val | val^2]
                  rloc2_sb = apool.tile([128, VSHP // 128, E2], BF)
                  with tc.tile_pool(name="psT", bufs=2, space="PSUM") as psT:
                      # dummy junk matmul: carries the psA->psT PSUM drain wait
                      dtp = psT.tile([64, 64], F32, tag="tp")
                      nc.tensor.matmul(
                          dtp[:], wdt_sb[:, 0, :], wdt_sb[:, 0, :],
                          start=True, stop=True,
                      )
                      nc.vector.tensor_copy(ostage[0:64, NT - 1, :], dtp[:])
                      for v in range(VSHP // 128):
                          tp = psT.tile([128, 64], F32, tag="tp")
                          nc.tensor.transpose(
                              tp[:],
                              rtabT_sb[:, v * 128 : (v + 1) * 128],
                              ident2_sb[:64, :64],
                          )
                          nc.vector.tensor_copy(rloc2_sb[:, v, 0:R], tp[:])
                          nc.scalar.square(rloc2_sb[:, v, R:E2], tp[:])
                      nc.sync.dma_start(
                          rloc2.rearrange("(v p) n -> p v n", p=128), rloc2_sb[:]
                      )

                      # ---- Phase B: AllGather rloc2 -> rtab2 ----
                      nc.gpsimd.collective_compute(
                          "AllGather",
                          ALU.bypass,
                          replica_groups=[list(range(NCORES))],
                          ins=[rloc2.opt()],
                          outs=[rtab2.opt()],
                      )

                      # ---- Phase C: gather + pool + FC ----
                      _phase_c(nc, tc, psT, rtab2, idx_sb, aux_sb, wret_sb,
                               biasr_sb, identb_sb, wdt_sb, ostage)

                      nc.sync.dma_start(
                          out.rearrange("(t p) n -> p t n", p=128), ostage[:]
                      )

    # Bacc's compile pipeline handles wait-limit lowering
    # (move_matmul_waits_to_ldweights, event semaphores, regalloc, ...)
    nc.compile()
    return nc


_NC_CACHE = {}


def _get_nc():
    key = (os.environ.get("KREPS", "1"), os.environ.get("KSTAGE", "full"))
    if key not in _NC_CACHE:
        _NC_CACHE[key] = build_kernel()
    return _NC_CACHE[key]


def _prepare(text_embeddings, kgl2token, W_down, W_re, b_re):
    import ml_dtypes

    emb = np.ascontiguousarray(np.asarray(text_embeddings, dtype=np.float32))
    ids = np.asarray(kgl2token)
    wd = np.asarray(W_down, dtype=np.float32)
    wr = np.asarray(W_re, dtype=np.float32)
    br = np.asarray(b_re, dtype=np.float32)

    # host-side scalars: lengths and scale factors (global mean over all rows)
    lengths = (ids > 0).sum(axis=1).astype(np.float32)  # [B]
    scale = np.log(lengths + 0.0)
    scale = scale / (scale.mean() + 1e-10)
    iscale = 1.0 / np.clip(scale, 0.01, None)
    invl = (1.0 / (lengths + 1e-10)).astype(np.float32)

    # remap ids into padded vocab layout
    ids64 = ids.astype(np.int64)
    rid = (ids64 // VSH) * VSHP + (ids64 % VSH)  # [B, S] < 32768

    wdt = np.ascontiguousarray(wd.T)  # [4096, 64]

    # W_re: result index = feat*3 + k  ->  W_k = W_re[:, k::3]  [64, 256]
    wret = np.zeros((2, 128, 3 * R), dtype=np.float32)
    for k in range(3):
        wkT = np.ascontiguousarray(wr[:, k::3].T)  # [256, 64]
        for kc in range(2):
            wret[kc, :, k * R : (k + 1) * R] = wkT[kc * 128 : (kc + 1) * 128, :]
    wret = wret.astype(ml_dtypes.bfloat16)
    biasr = np.tile(br[None, :], (128, 1)).astype(np.float32)
    identm = np.eye(128, dtype=np.float32)

    in_maps = []
    for c in range(NCORES):
        embt = np.zeros((HID, VSHP), dtype=np.float32)
        embt[:, :VSH] = emb[c * VSH : (c + 1) * VSH, :].T
        # per-core padded rows
        rid_c = np.zeros((BPAD, S), dtype=np.int64)
        rid_c[:BSH] = rid[c * BSH : (c + 1) * BSH]
        # gather order: j = t*2048 + s*128 + r
        L = rid_c.reshape(NT, 128, S).transpose(0, 2, 1).reshape(-1)  # [BPAD*S]
        idx16 = L.reshape(-1, 16).T.astype(np.int16)  # [16, BPAD]
        idxsb = np.ascontiguousarray(np.tile(idx16, (8, 1)))  # [128, BPAD]

        auxc = np.zeros((128, 3 * NT), dtype=np.float32)
        for name_i, v in enumerate((invl, scale, iscale)):
            vc = np.ones(BPAD, dtype=np.float32)
            vc[:BSH] = v[c * BSH : (c + 1) * BSH]
            auxc[:, name_i * NT : (name_i + 1) * NT] = vc.reshape(NT, 128).T
        in_maps.append(
            dict(embt=embt, wdt=wdt, idx=idxsb, aux=auxc, wret=wret,
                 biasr=biasr, ident=identm)
        )
    return in_maps, lengths, scale, iscale, invl


def _patch_rows(result, text_embeddings, kgl2token, W_down, W_re, b_re,
                scale_all, iscale_all, invl_all):
    """Recompute rows containing any id==0 token exactly (host, numpy)."""
    ids = np.asarray(kgl2token)
    bad = np.nonzero((ids <= 0).any(axis=1))[0]
    if len(bad) == 0:
        return result
    emb = np.asarray(text_embeddings, dtype=np.float32)
    wd = np.asarray(W_down, dtype=np.float32)
    wr = np.asarray(W_re, dtype=np.float32)
    br = np.asarray(b_re, dtype=np.float32)
    for r in bad:
        tok_ids = ids[r].astype(np.int64)
        tok = emb[tok_ids] @ wd.T  # [S, R]
        mask = (tok_ids > 0).astype(np.float32)[:, None]
        length = mask.sum()
        masked = tok * mask
        mean = masked.sum(axis=0) / (length + 1e-10)
        sq_mean = (tok * tok * mask).sum(axis=0) / (length + 1e-10)
        mx = (masked + (1.0 - mask) * (-1e10)).max(axis=0)
        mn = (masked + (1.0 - mask) * (1e10)).min(axis=0)
        std = np.sqrt(np.clip(sq_mean - mean * mean, 1e-6, None))
        features = np.concatenate([mean, mx, mn, std])  # [256]
        scales = np.array([1.0, scale_all[r], iscale_all[r]], dtype=np.float32)
        flat = (features[:, None] * scales[None, :]).reshape(-1)  # [768]
        res = flat @ wr.T + br
        nrm = np.linalg.norm(res)
        result[r] = res / max(nrm, 1e-12)
    return result


def kernel(text_embeddings, kgl2token, W_down, W_re, b_re, _trace=False):
    nc = _get_nc()
    in_maps, lengths, scale, iscale, invl = _prepare(
        text_embeddings, kgl2token, W_down, W_re, b_re
    )
    r = run_bass_kernel_spmd(nc, in_maps, core_ids=list(range(NCORES)), trace=_trace)
    outs = [r.results[c]["out"][:BSH] for c in range(NCORES)]
    result = np.concatenate(outs, axis=0).astype(np.float32)
    result = _patch_rows(
        result, text_embeddings, kgl2token, W_down, W_re, b_re, scale, iscale, invl
    )
    if _trace:
        return result, r
    return result


# revision 11
# speedup vs baseline: 1.7140x; 1.2671x over previous
"""BasePNARetriever Trainium2 kernel (8 NeuronCores, SPMD).

Strategy (v2):
  - Vocab-shard the big embedding @ W_down.T matmul: each core streams a
    [4096, 4096] (host-transposed, padded) slice of text_embeddings and
    computes RtabT[64, 4096] on PE, accumulating in PSUM over 32 K-chunks.
  - PE-transpose RtabT back to row-major and emit a bf16 table with each
    row packed as [val(64) | val^2(64)] (256B rows): the squares ride along
    for free in the gather, eliminating the per-chunk ACT square pass.
    AllGather the 1MB local slice into the full rtab2[32768, 128] bf16.
  - Each core dma_gathers its (padded) rows x 16 tokens = 100352 vectors of
    256B from rtab2, round-robining the 4 SWDGE queues so descriptor
    generation runs on all four Q7 core-pairs concurrently (the Q7 desc-gen
    at ~8ns/desc is otherwise the phase C wall).  Pooling runs as chunk-wide
    bf16 binary trees on DVE (sum+sumsq fused over the packed 128-elem
    rows), per-row scalar finals split between ACT (scale-activations) and
    DVE, small FC on PE via PE-transposed bf16 features, L2-normalize.
  - Host precomputes lengths/log-scales (includes a global mean over all
    50000 rows) and patches the rare rows containing id==0 tokens (the
    device path ignores the valid-token mask; ~25 rows in 50000).

Sync-architecture notes (walrus limits): a Matmult may carry at most ONE
sync wait; HWDGE (nc.sync) DMAs are also tightly limited; DVE/ACT/Pool
queue instructions tolerate several.  Hence: emb chunk loads go through
SWDGE (nc.gpsimd), every PE dummy/absorber reads only wdt_sb (whose DMA
lane PE observes on its first matmul), an extra junk matmul into the psA
pad columns absorbs the PSUM drain wait at the psA->psT transition, and
phase C reuses the still-open psT pool (psF=psT) so PSUM bank history is
already PE-observed when the feature transposes start.
"""

import sys

sys.path.insert(0, "/opt/trn_rl_repo")

import os

import numpy as np

import concourse.bass as bass
import concourse.bacc as bacc
import concourse.mybir as mybir
import concourse.tile as tile
from concourse.bass_utils import run_bass_kernel_spmd

F32 = mybir.dt.float32
BF = mybir.dt.bfloat16
I16 = mybir.dt.int16
AF = mybir.ActivationFunctionType
ALU = mybir.AluOpType

NCORES = 8
VOCAB, HID, R, B, S = 32000, 4096, 64, 50000, 16
VSH = VOCAB // NCORES          # 4000 real vocab rows per core
VSHP = 4096                    # padded vocab rows per core (32 x 128)
VOCABP = VSHP * NCORES         # 32768 padded vocab
KC = HID // 128                # 32 contraction chunks
BSH = B // NCORES              # 6250 rows per core
NT = 49                        # row tiles of 128 (6272 padded rows)
BPAD = NT * 128                # 6272
E2 = 2 * R                     # 128: packed table row [val(64)|sq(64)]
CH_T = 2                       # row-tiles per gather chunk
CHUNKS = [(i, min(CH_T, NT - i)) for i in range(0, NT, CH_T)]  # (tile0, ntiles)


def _phase_c(nc, tc, psT, rtab2, idx_sb, aux_sb, wret_sb, biasr_sb, identb_sb,
             wdt_sb, ostage):
    with (
        tc.tile_pool(name="g", bufs=6) as gpool,
        tc.tile_pool(name="tr", bufs=1) as tpool,
        tc.tile_pool(name="f", bufs=2) as fpool,
        tc.tile_pool(name="psG", bufs=1, space="PSUM") as psG,
    ):
        psF = psT  # reuse the open pool: bank history already PE-observed

        stage = os.environ.get("KSTAGE", "full")
        for ci, (t0, ntile) in enumerate(CHUNKS):
            nidx = ntile * 2048
            nslot = ntile * 16
            g = gpool.tile([128, CH_T * 16, E2], BF, tag="g")
            if stage in ("gather", "full"):
                nc.gpsimd.dma_gather(
                    g[:, :nslot, :],
                    rtab2[:],
                    idx_sb[:, t0 * 128 : t0 * 128 + nidx // 16],
                    nidx,
                    nidx,
                    E2,
                    single_packet=False,
                    queue_num=ci % 4,
                )
            if stage != "full":
                continue

            g4 = g[:, :nslot, :].rearrange("p (t s) e -> p t s e", s=16)

            # -- chunk-wide bf16 trees over the 16 tokens --
            # add-tree runs on the full packed rows: sums values AND squares
            a1 = tpool.tile([128, CH_T, 8, E2], BF, tag="a1")
            nc.vector.tensor_tensor(
                a1[:, :ntile], g4[:, :, 0:8, :], g4[:, :, 8:16, :], ALU.add)
            a2 = tpool.tile([128, CH_T, 4, E2], BF, tag="a2")
            nc.vector.tensor_tensor(
                a2[:, :ntile], a1[:, :ntile, 0:4], a1[:, :ntile, 4:8], ALU.add)
            a3 = tpool.tile([128, CH_T, 2, E2], BF, tag="a3")
            nc.vector.tensor_tensor(
                a3[:, :ntile], a2[:, :ntile, 0:2], a2[:, :ntile, 2:4], ALU.add)
            addf = tpool.tile([128, CH_T, E2], F32, tag="addf")
            nc.vector.tensor_tensor(
                addf[:, :ntile], a3[:, :ntile, 0], a3[:, :ntile, 1], ALU.add)

            featc = fpool.tile([128, CH_T, 4 * R], BF, tag="featc")

            def mmtree(op, dst_lo):
                b1 = tpool.tile([128, CH_T, 8, R], BF, tag=f"b1{dst_lo}")
                nc.vector.tensor_tensor(
                    b1[:, :ntile], g4[:, :, 0:8, 0:R], g4[:, :, 8:16, 0:R], op)
                b2 = tpool.tile([128, CH_T, 4, R], BF, tag=f"b2{dst_lo}")
                nc.vector.tensor_tensor(
                    b2[:, :ntile], b1[:, :ntile, 0:4], b1[:, :ntile, 4:8], op)
                b3 = tpool.tile([128, CH_T, 2, R], BF, tag=f"b3{dst_lo}")
                nc.vector.tensor_tensor(
                    b3[:, :ntile], b2[:, :ntile, 0:2], b2[:, :ntile, 2:4], op)
                nc.vector.tensor_tensor(
                    featc[:, :ntile, dst_lo : dst_lo + R],
                    b3[:, :ntile, 0], b3[:, :ntile, 1], op)

            mmtree(ALU.max, R)       # max -> featc[:, :, 64:128]
            mmtree(ALU.min, 2 * R)   # min -> featc[:, :, 128:192]

            ss = fpool.tile([128, CH_T], F32, tag="ss")

            for tt in range(ntile):
                t = t0 + tt
                invl = aux_sb[:, t : t + 1]
                sc = aux_sb[:, NT + t : NT + t + 1]
                isc = aux_sb[:, 2 * NT + t : 2 * NT + t + 1]

                # mean / std (fp32 chain on ACT + DVE)
                m = fpool.tile([128, R], F32, tag="m")
                nc.scalar.activation(m[:], addf[:, tt, 0:R], AF.Copy, scale=invl)
                nc.scalar.copy(featc[:, tt, 0:R], m[:])
                sqm = fpool.tile([128, R], F32, tag="sqm")
                nc.scalar.activation(
                    sqm[:], addf[:, tt, R:E2], AF.Copy, scale=invl)
                msq = fpool.tile([128, R], F32, tag="msq")
                nc.scalar.square(msq[:], m[:])
                nc.vector.tensor_tensor(sqm[:], sqm[:], msq[:], ALU.subtract)
                nc.vector.tensor_scalar_max(sqm[:], sqm[:], 1e-6)
                nc.scalar.sqrt(featc[:, tt, 3 * R : 4 * R], sqm[:])

                # FC: G_k = features @ W_k.T via PE-transposed features
                fts = []
                for kc in range(2):
                    ftp = psF.tile([128, 128], BF, tag="ftp")
                    nc.tensor.transpose(
                        ftp[:], featc[:, tt, kc * 128 : (kc + 1) * 128],
                        identb_sb[:],
                    )
                    ft = fpool.tile([128, 128], BF, tag=f"fts{kc}")
                    nc.scalar.activation(ft[:], ftp[:], AF.Copy)
                    fts.append(ft)
                gps = [
                    psG.tile([128, R], F32, tag=f"gp{k}", name=f"gp{k}_{t}")
                    for k in range(3)
                ]
                for kc in range(2):
                    for k in range(3):
                        nc.tensor.matmul(
                            gps[k][:],
                            fts[kc][:],
                            wret_sb[:, kc, k * R : (k + 1) * R],
                            start=(kc == 0),
                            stop=(kc == 1),
                        )
                # res = G0 + G1*scale + G2*iscale + bias
                r1 = fpool.tile([128, R], F32, tag="r1")
                nc.scalar.activation(r1[:], gps[1][:], AF.Copy, scale=sc)
                r2 = fpool.tile([128, R], F32, tag="r2")
                nc.scalar.activation(r2[:], gps[2][:], AF.Copy, scale=isc)
                res = fpool.tile([128, R], F32, tag="res")
                nc.vector.tensor_add(res[:], gps[0][:], r1[:])
                nc.vector.tensor_add(res[:], res[:], r2[:])
                nc.vector.tensor_add(ostage[:, t, :], res[:], biasr_sb[:])
                # L2 norm: squares + row-sum in one ACT op
                sqr = fpool.tile([128, R], F32, tag="sqr")
                nc.scalar.activation(
                    sqr[:], ostage[:, t, :], AF.Square,
                    accum_out=ss[:, tt : tt + 1])

            # normalize per chunk: ostage[:, t, :] *= 1/max(sqrt(ss), eps)
            nrm = fpool.tile([128, CH_T], F32, tag="nrm")
            nc.scalar.sqrt(nrm[:, :ntile], ss[:, :ntile])
            nc.vector.tensor_scalar_max(nrm[:, :ntile], nrm[:, :ntile], 1e-12)
            rin = fpool.tile([128, CH_T], F32, tag="rin")
            nc.vector.reciprocal(rin[:, :ntile], nrm[:, :ntile])
            for tt in range(ntile):
                t = t0 + tt
                nc.vector.tensor_scalar_mul(
                    ostage[:, t, :], ostage[:, t, :], rin[:, tt : tt + 1])


def build_kernel():
    nc = bacc.Bacc(
        "TRN2",
        target_bir_lowering=False,
        debug=False,
        num_devices=NCORES,
        num_swdge_queues=4,
    )
    embt = nc.declare_dram_parameter("embt", [HID, VSHP], F32, isOutput=False)
    wdt = nc.declare_dram_parameter("wdt", [HID, R], F32, isOutput=False)
    idx = nc.declare_dram_parameter("idx", [128, BPAD], I16, isOutput=False)
    aux = nc.declare_dram_parameter("aux", [128, 3 * NT], F32, isOutput=False)
    wret = nc.declare_dram_parameter("wret", [2, 128, 3 * R], BF, isOutput=False)
    biasr = nc.declare_dram_parameter("biasr", [128, R], F32, isOutput=False)
    ident = nc.declare_dram_parameter("ident", [128, 128], F32, isOutput=False)
    out = nc.declare_dram_parameter("out", [BPAD, R], F32, isOutput=True)

    with tile.TileContext(nc) as tc:
        with (
            tc.tile_pool(name="dram", bufs=1, space="DRAM") as dpool,
            tc.tile_pool(name="const", bufs=1) as cpool,
        ):
            rloc2 = dpool.tile([VSHP, E2], BF)
            rtab2 = dpool.tile([VOCABP, E2], BF)

            wdt_sb = cpool.tile([128, KC, R], F32)
            nc.sync.dma_start(wdt_sb[:], wdt.rearrange("(k p) n -> p k n", p=128))
            idx_sb = cpool.tile([128, BPAD], I16)
            nc.sync.dma_start(idx_sb[:], idx[:])
            aux_sb = cpool.tile([128, 3 * NT], F32)
            nc.sync.dma_start(aux_sb[:], aux[:])
            wret_raw = cpool.tile([128, 2, 3 * R], BF)
            nc.sync.dma_start(wret_raw[:], wret.rearrange("c p n -> p c n"))
            wret_sb = cpool.tile([128, 2, 3 * R], BF)
            nc.vector.tensor_copy(wret_sb[:], wret_raw[:])
            biasr_sb = cpool.tile([128, R], F32)
            nc.sync.dma_start(biasr_sb[:], biasr[:])
            ident_sb = cpool.tile([128, 128], F32)
            nc.sync.dma_start(ident_sb[:], ident[:])
            ostage = cpool.tile([128, NT, R], F32)

            # identity staged through DVE so PE transposes dep on DVE sem only
            ident2_sb = cpool.tile([128, 128], F32)
            nc.vector.tensor_copy(ident2_sb[:], ident_sb[:])
            identb_sb = cpool.tile([128, 128], BF)
            nc.vector.tensor_copy(identb_sb[:], ident_sb[:])

            # ---- Phase A: RtabT = W_downT.T @ embT ----
            # KREPS>1 repeats the whole pipeline for launch-overhead-free
            # wall-clock measurement ((T(N)-T(1))/(N-1) = per-rep time).
            for _rep in range(int(os.environ.get("KREPS", "1"))):
              with (
                  tc.tile_pool(name="emb", bufs=3) as epool,
                  tc.tile_pool(name="stageA", bufs=1) as apool,
              ):
                  rtabT_sb = apool.tile([64, VSHP], F32)
                  with tc.tile_pool(name="psA", bufs=1, space="PSUM") as psA:
                      rtabT_ps = psA.tile([64, VSHP], F32)
                      # gate: junk matmul reading only wdt_sb -> absorbs the wdt
                      # DMA-lane wait so real matmuls carry just their ech lane
                      nc.tensor.matmul(
                          rtabT_ps[:, VSHP - 64 : VSHP - 32],
                          wdt_sb[:, 0, :],
                          wdt_sb[:, 0, 0:32],
                          start=True,
                          stop=True,
                          skip_group_check=True,
                      )
                      for k in range(KC):
                          ech = epool.tile([128, VSHP], F32, tag="ech")
                          nc.gpsimd.dma_start(ech[:], embt[k * 128 : (k + 1) * 128, :])
                          for vb in range(VSHP // 512):
                              c0 = vb * 512
                              c1 = min((vb + 1) * 512, VSHP - 64)
                              nc.tensor.matmul(
                                  rtabT_ps[:, c0:c1],
                                  wdt_sb[:, k, :],
                                  ech[:, c0:c1],
                                  start=(k == 0),
                                  stop=(k == KC - 1),
                              )
                      # absorber: junk matmul into the other pad half; its only
                      # wait is the PSUM drain (PE self-sem), freeing later
                      # matmuls from carrying it (Matmult = 1 wait max)
                      nc.tensor.matmul(
                          rtabT_ps[:, VSHP - 32 : VSHP],
                          wdt_sb[:, 0, :],
                          wdt_sb[:, 0, 32:64],
                          start=True,
                          stop=True,
                          skip_group_check=True,
                      )
                      nc.vector.tensor_copy(rtabT_sb[:], rtabT_ps[:])

                  # bf16 table slice, rows packed [val | val^2]
                  rloc2_sb = apool.tile([128, VSHP // 128, E2], BF)
                  with tc.tile_pool(name="psT", bufs=2, space="PSUM") as psT:
                      # dummy junk matmul: carries the psA->psT PSUM drain wait
                      dtp = psT.tile([64, 64], F32, tag="tp")
                      nc.tensor.matmul(
                          dtp[:], wdt_sb[:, 0, :], wdt_sb[:, 0, :],
                          start=True, stop=True,
                      )
                      nc.vector.tensor_copy(ostage[0:64, NT - 1, :], dtp[:])
                      for v in range(VSHP // 128):
                          tp = psT.tile([128, 64], F32, tag="tp")
                          nc.tensor.transpose(
                              tp[:],
                              rtabT_sb[:, v * 128 : (v + 1) * 128],
                              ident2_sb[:64, :64],
                          )
                          nc.vector.tensor_copy(rloc2_sb[:, v, 0:R], tp[:])
                          nc.scalar.square(rloc2_sb[:, v, R:E2], tp[:])
                      nc.sync.dma_start(
                          rloc2.rearrange("(v p) n -> p v n", p=128), rloc2_sb[:]
                      )

                      # ---- Phase B: AllGather rloc2 -> rtab2 ----
                      nc.gpsimd.collective_compute(
                          "AllGather",
                          ALU.bypass,
                          replica_groups=[list(range(NCORES))],
                          ins=[rloc2.opt()],
                          outs=[rtab2.opt()],
                      )

                      # ---- Phase C: gather + pool + FC ----
                      _phase_c(nc, tc, psT, rtab2, idx_sb, aux_sb, wret_sb,
                               biasr_sb, identb_sb, wdt_sb, ostage)

                      nc.sync.dma_start(
                          out.rearrange("(t p) n -> p t n", p=128), ostage[:]
                      )

    # Bacc's compile pipeline handles wait-limit lowering
    # (move_matmul_waits_to_ldweights, event semaphores, regalloc, ...)
    nc.compile()
    return nc


_NC_CACHE = {}


def _get_nc():
    key = (os.environ.get("KREPS", "1"), os.environ.get("KSTAGE", "full"))
    if key not in _NC_CACHE:
        _NC_CACHE[key] = build_kernel()
    return _NC_CACHE[key]


def _prepare(text_embeddings, kgl2token, W_down, W_re, b_re):
    import ml_dtypes

    emb = np.ascontiguousarray(np.asarray(text_embeddings, dtype=np.float32))
    ids = np.asarray(kgl2token)
    wd = np.asarray(W_down, dtype=np.float32)
    wr = np.asarray(W_re, dtype=np.float32)
    br = np.asarray(b_re, dtype=np.float32)

    # host-side scalars: lengths and scale factors (global mean over all rows)
    lengths = (ids > 0).sum(axis=1).astype(np.float32)  # [B]
    scale = np.log(lengths + 0.0)
    scale = scale / (scale.mean() + 1e-10)
    iscale = 1.0 / np.clip(scale, 0.01, None)
    invl = (1.0 / (lengths + 1e-10)).astype(np.float32)

    # remap ids into padded vocab layout
    ids64 = ids.astype(np.int64)
    rid = (ids64 // VSH) * VSHP + (ids64 % VSH)  # [B, S] < 32768

    wdt = np.ascontiguousarray(wd.T)  # [4096, 64]

    # W_re: result index = feat*3 + k  ->  W_k = W_re[:, k::3]  [64, 256]
    wret = np.zeros((2, 128, 3 * R), dtype=np.float32)
    for k in range(3):
        wkT = np.ascontiguousarray(wr[:, k::3].T)  # [256, 64]
        for kc in range(2):
            wret[kc, :, k * R : (k + 1) * R] = wkT[kc * 128 : (kc + 1) * 128, :]
    wret = wret.astype(ml_dtypes.bfloat16)
    biasr = np.tile(br[None, :], (128, 1)).astype(np.float32)
    identm = np.eye(128, dtype=np.float32)

    in_maps = []
    for c in range(NCORES):
        embt = np.zeros((HID, VSHP), dtype=np.float32)
        embt[:, :VSH] = emb[c * VSH : (c + 1) * VSH, :].T
        # per-core padded rows
        rid_c = np.zeros((BPAD, S), dtype=np.int64)
        rid_c[:BSH] = rid[c * BSH : (c + 1) * BSH]
        # gather order: j = t*2048 + s*128 + r
        L = rid_c.reshape(NT, 128, S).transpose(0, 2, 1).reshape(-1)  # [BPAD*S]
        idx16 = L.reshape(-1, 16).T.astype(np.int16)  # [16, BPAD]
        idxsb = np.ascontiguousarray(np.tile(idx16, (8, 1)))  # [128, BPAD]

        auxc = np.zeros((128, 3 * NT), dtype=np.float32)
        for name_i, v in enumerate((invl, scale, iscale)):
            vc = np.ones(BPAD, dtype=np.float32)
            vc[:BSH] = v[c * BSH : (c + 1) * BSH]
            auxc[:, name_i * NT : (name_i + 1) * NT] = vc.reshape(NT, 128).T
        in_maps.append(
            dict(embt=embt, wdt=wdt, idx=idxsb, aux=auxc, wret=wret,
                 biasr=biasr, ident=identm)
        )
    return in_maps, lengths, scale, iscale, invl


def _patch_rows(result, text_embeddings, kgl2token, W_down, W_re, b_re,
                scale_all, iscale_all, invl_all):
    """Recompute rows containing any id==0 token exactly (host, numpy)."""
    ids = np.asarray(kgl2token)
    bad = np.nonzero((ids <= 0).any(axis=1))[0]
    if len(bad) == 0:
        return result
    emb = np.asarray(text_embeddings, dtype=np.float32)
    wd = np.asarray(W_down, dtype=np.float32)
    wr = np.asarray(W_re, dtype=np.float32)
    br = np.asarray(b_re, dtype=np.float32)
    for r in bad:
        tok_ids = ids[r].astype(np.int64)
        tok = emb[tok_ids] @ wd.T  # [S, R]
        mask = (tok_ids > 0).astype(np.float32)[:, None]
        length = mask.sum()
        masked = tok * mask
        mean = masked.sum(axis=0) / (length + 1e-10)
        sq_mean = (tok * tok * mask).sum(axis=0) / (length + 1e-10)
        mx = (masked + (1.0 - mask) * (-1e10)).max(axis=0)
        mn = (masked + (1.0 - mask) * (1e10)).min(axis=0)
        std = np.sqrt(np.clip(sq_mean - mean * mean, 1e-6, None))
        features = np.concatenate([mean, mx, mn, std])  # [256]
        scales = np.array([1.0, scale_all[r], iscale_all[r]], dtype=np.float32)
        flat = (features[:, None] * scales[None, :]).reshape(-1)  # [768]
        res = flat @ wr.T + br
        nrm = np.linalg.norm(res)
        result[r] = res / max(nrm, 1e-12)
    return result


def kernel(text_embeddings, kgl2token, W_down, W_re, b_re, _trace=False):
    nc = _get_nc()
    in_maps, lengths, scale, iscale, invl = _prepare(
        text_embeddings, kgl2token, W_down, W_re, b_re
    )
    r = run_bass_kernel_spmd(nc, in_maps, core_ids=list(range(NCORES)), trace=_trace)
    outs = [r.results[c]["out"][:BSH] for c in range(NCORES)]
    result = np.concatenate(outs, axis=0).astype(np.float32)
    result = _patch_rows(
        result, text_embeddings, kgl2token, W_down, W_re, b_re, scale, iscale, invl
    )
    if _trace:
        return result, r
    return result
